# revision 5
# baseline (speedup 1.0000x reference)
"""ConceptMoE forward on 8 trn2 NeuronCores.

Strategy (B=1):
- Embedding gather + h-transpose replicated on all cores (memory-light).
- ChunkModule q/k projections sequence-sharded (1/8 of positions per core,
  fp32 matmuls to preserve the bp>=0.5 mask decisions), cos exchanged via a
  tiny AllGather.
- Dechunk EMA scan replicated, computed with the native DVE linear-recurrence
  instruction (tensor_tensor_scan) in hidden-major layout.
- lm_head vocab-sharded (4000 vocab rows per core) in float32r (full PE speed,
  ~1e-4 relative error).

All model scales (ENC 16, CPT 4^23, DEC 16) are folded into the scan
coefficients and the final h-combine, so the gathered embeddings flow through
unscaled and cos (scale-invariant) is computed from raw h.
"""

import numpy as np

import concourse.bass as bass
import concourse.mybir as mybir
import concourse.tile as tile
from concourse.bass_utils import run_bass_kernel_spmd
from concourse.masks import make_identity

P = 128
L = 4096
HD = 2048
V = 32000
NCORES = 8
SEQ = L // NCORES  # 512 cos positions per core
VS = V // NCORES  # 4000 vocab rows per core
QKL = SEQ + P  # 640 gathered rows per core for q/k (513 used)
CPT_SCALE = 4.0**23
OUT_SCALE = 256.0  # ENC(16) * DEC(16) folded to the end

F32 = mybir.dt.float32
F32R = mybir.dt.float32r
I32 = mybir.dt.int32
MULT = mybir.AluOpType.mult
ADD = mybir.AluOpType.add


def _legalize_multi_waits(nc):
    """This walrus build accepts at most one sync wait per instruction; split
    extra waits onto same-engine NOPs placed immediately before."""
    count = [0]
    for f in nc.m.functions:
        for bb in f.blocks:
            insts = bb.instructions
            if not any(
                ins.sync_info is not None
                and ins.sync_info.on_wait
                and len(ins.sync_info.on_wait) > 1
                for ins in insts
            ):
                continue
            new_insts = []
            for ins in insts:
                si = ins.sync_info
                if si is not None and si.on_wait and len(si.on_wait) > 1:
                    waits = list(si.on_wait)
                    for w in waits[:-1]:
                        count[0] += 1
                        nop = mybir.InstNoOp(
                            name=f"legal-wait-nop-{count[0]}",
                            engine=ins.engine,
                            ins=[],
                            outs=[],
                        )
                        nop.sync_info = mybir.SyncInfo(on_wait=[w], on_update=[])
                        new_insts.append(nop)
                    ins.sync_info = mybir.SyncInfo(
                        on_wait=[waits[-1]], on_update=list(si.on_update or [])
                    )
                new_insts.append(ins)
            bb.instructions = new_insts


def build_program():
    nc = bass.Bass()

    table = nc.declare_dram_parameter("table", [L, HD], F32, isOutput=False)
    remap = nc.declare_dram_parameter("remap", [L], I32, isOutput=False)
    qk_ids = nc.declare_dram_parameter("qk_ids", [QKL], I32, isOutput=False)
    q_wT = nc.declare_dram_parameter("q_wT", [HD, HD], F32, isOutput=False)
    k_wT = nc.declare_dram_parameter("k_wT", [HD, HD], F32, isOutput=False)
    lm_wT = nc.declare_dram_parameter("lm_wT", [HD, VS], F32, isOutput=False)
    logits = nc.declare_dram_parameter("logits", [L, VS], F32, isOutput=True)
    aux_out = nc.declare_dram_parameter("aux", [1, 1], F32, isOutput=True)

    NSEQ = L // P  # 32 seq tiles
    NK = HD // P  # 16 hidden tiles
    NMQ = QKL // P  # 5 local seq tiles
    NM = SEQ // P  # 4 qk output tiles

    with tile.TileContext(nc) as tc:
        with (
            tc.tile_pool(name="persist", bufs=1) as pp,
            tc.tile_pool(name="dram", bufs=1, space="DRAM") as dram,
        ):
            hT = dram.tile([HD, L], F32)
            ohT = dram.tile([HD, L], F32R)
            cc_in = dram.tile([NM, P], F32)
            cc_out = dram.tile([NCORES * NM, P], F32)

            ident = pp.tile([P, P], F32)
            make_identity(nc, ident[:])
            cos_sb = pp.tile([NM, P], F32, tag="cos_sb")

            # ========== stages A-qk + B share the hT_loc pool ==========
            with tc.tile_pool(name="hloc", bufs=1) as hp:
                hT_loc = [
                    hp.tile([P, QKL], F32, tag=f"hTloc{k}", name=f"hTloc{k}")
                    for k in range(NK)
                ]

                # ---- stage A-qk: local gather + transpose ----
                with (
                    tc.tile_pool(name="aqk", bufs=1) as ap,
                    tc.tile_pool(name="aqk_ps", bufs=2, space="PSUM") as aps,
                ):
                    for mt in range(NMQ):
                        idt = ap.tile([P, 1], I32, tag="idt", bufs=2)
                        nc.sync.dma_start(
                            out=idt[:], in_=qk_ids[mt * P : (mt + 1) * P, None]
                        )
                        g = ap.tile([P, HD], F32, tag="g", bufs=2)
                        nc.gpsimd.indirect_dma_start(
                            out=g[:],
                            out_offset=None,
                            in_=table[:],
                            in_offset=bass.IndirectOffsetOnAxis(ap=idt[:, :1], axis=0),
                        )
                        for k in range(NK):
                            tps = aps.tile([P, P], F32, tag="tps")
                            nc.tensor.transpose(
                                out=tps[:],
                                in_=g[:, k * P : (k + 1) * P],
                                identity=ident[:],
                            )
                            nc.vector.tensor_copy(
                                hT_loc[k][:, mt * P : (mt + 1) * P], tps[:]
                            )

                # ---- stage B: q/k fp32 projections + cos ----
                with (
                    tc.tile_pool(name="bqk", bufs=1) as bp_,
                    tc.tile_pool(name="bqk_ps", bufs=1, space="PSUM") as bps,
                ):
                    proj_sb = {
                        "q": [
                            bp_.tile([P, HD], F32, tag=f"q{m}", name=f"q{m}")
                            for m in range(NM)
                        ],
                        "k": [
                            bp_.tile([P, HD], F32, tag=f"k{m}", name=f"k{m}")
                            for m in range(NM)
                        ],
                    }
                    for name, wsrc, off in [("q", q_wT, 0), ("k", k_wT, 1)]:
                        for n in range(HD // 512):
                            w = bp_.tile([P, NK * 512], F32, tag="wqk", bufs=2)
                            nc.sync.dma_start(
                                out=w[:].rearrange("p (a b) -> p a b", a=NK),
                                in_=wsrc.rearrange("(a p) n -> p a n", p=P)[
                                    :, :, n * 512 : (n + 1) * 512
                                ],
                            )
                            for m in range(NM):
                                ps = bps.tile([P, 512], F32, tag="pqk", bufs=2)
                                for k in range(NK):
                                    nc.tensor.matmul(
                                        ps[:],
                                        hT_loc[k][:, m * P + off : (m + 1) * P + off],
                                        w[:, k * 512 : (k + 1) * 512],
                                        start=(k == 0),
                                        stop=(k == NK - 1),
                                    )
                                nc.vector.tensor_copy(
                                    proj_sb[name][m][:, n * 512 : (n + 1) * 512],
                                    ps[:],
                                )

                    numc = bp_.tile([P, NM], F32, tag="numc")
                    nqc = bp_.tile([P, NM], F32, tag="nqc")
                    nkc = bp_.tile([P, NM], F32, tag="nkc")
                    scr = bp_.tile([P, HD], F32, tag="scr")
                    for m in range(NM):
                        qm, km = proj_sb["q"][m], proj_sb["k"][m]
                        nc.vector.scalar_tensor_tensor(
                            out=scr[:], in0=qm[:], scalar=1.0, in1=km[:],
                            op0=MULT, op1=MULT, accum_out=numc[:, m : m + 1],
                        )
                        nc.vector.scalar_tensor_tensor(
                            out=scr[:], in0=qm[:], scalar=1.0, in1=qm[:],
                            op0=MULT, op1=MULT, accum_out=nqc[:, m : m + 1],
                        )
                        nc.vector.scalar_tensor_tensor(
                            out=scr[:], in0=km[:], scalar=1.0, in1=km[:],
                            op0=MULT, op1=MULT, accum_out=nkc[:, m : m + 1],
                        )
                    nn = bp_.tile([P, NM], F32, tag="nn")
                    nc.vector.tensor_mul(nn[:], nqc[:], nkc[:])
                    nc.scalar.sqrt(nn[:], nn[:])
                    nc.vector.tensor_scalar_max(nn[:], nn[:], 1e-24)
                    nc.vector.reciprocal(nn[:], nn[:])
                    cosc = bp_.tile([P, NM], F32, tag="cosc")
                    nc.vector.tensor_mul(cosc[:], numc[:], nn[:])
                    # [P, NM] -> [NM, P] so flat order is (m, p) = position
                    cps = bps.tile([P, P], F32, tag="ctp")
                    nc.tensor.transpose(
                        out=cps[:NM, :], in_=cosc[:], identity=ident[:]
                    )
                    nc.vector.tensor_copy(cos_sb[:], cps[:NM, :])
                nc.sync.dma_start(out=cc_in[:], in_=cos_sb[:])

            # ---- stage A-full: full gather + transpose to hT (DRAM) ----
            with (
                tc.tile_pool(name="afull", bufs=1) as fp,
                tc.tile_pool(name="afull_ps", bufs=2, space="PSUM") as fps,
            ):
                for s in range(NSEQ):
                    fidt = fp.tile([P, 1], I32, tag="fidt", bufs=2)
                    nc.sync.dma_start(
                        out=fidt[:], in_=remap[s * P : (s + 1) * P, None]
                    )
                    fg = fp.tile([P, HD], F32, tag="fg", bufs=3)
                    nc.gpsimd.indirect_dma_start(
                        out=fg[:],
                        out_offset=None,
                        in_=table[:],
                        in_offset=bass.IndirectOffsetOnAxis(ap=fidt[:, :1], axis=0),
                    )
                    for k in range(NK):
                        ftps = fps.tile([P, P], F32, tag="ftps")
                        nc.tensor.transpose(
                            out=ftps[:],
                            in_=fg[:, k * P : (k + 1) * P],
                            identity=ident[:],
                        )
                        ftsb = fp.tile([P, P], F32, tag="ftsb", bufs=4)
                        nc.vector.tensor_copy(ftsb[:], ftps[:])
                        nc.sync.dma_start(
                            out=hT[k * P : (k + 1) * P, s * P : (s + 1) * P],
                            in_=ftsb[:],
                        )

            # ---- stage C: allgather cos -> bp, coefficient rows, aux ----
            nc.gpsimd.collective_compute(
                "AllGather",
                mybir.AluOpType.bypass,
                replica_groups=[list(range(NCORES))],
                ins=[cc_in[:]],
                outs=[cc_out[:]],
            )
            with tc.tile_pool(name="abc", bufs=1) as abc:
                a_bc = abc.tile([P, L], F32, tag="a_bc")
                coef_bc = abc.tile([P, L], F32, tag="coef_bc")
                with (
                    tc.tile_pool(name="cst", bufs=1) as cp,
                    tc.tile_pool(name="cst_ps", bufs=2, space="PSUM") as cps_,
                ):
                    ag = cp.tile([1, L], F32, tag="ag")
                    nc.sync.dma_start(
                        out=ag[:], in_=cc_out[:].rearrange("a b -> (a b)")[None, :]
                    )
                    bprow = cp.tile([1, L], F32, tag="bprow")
                    # bp[t] = clip((1-cos[t-1])/2, 0, 1); bp[0] = 1
                    nc.scalar.activation(
                        bprow[:, 1:L], ag[:, 0 : L - 1],
                        mybir.ActivationFunctionType.Copy, bias=0.0, scale=-0.5,
                    )
                    nc.vector.tensor_scalar(
                        bprow[:, 1:L], bprow[:, 1:L], 0.5, 0.0, ADD,
                        mybir.AluOpType.max,
                    )
                    nc.vector.tensor_scalar_min(bprow[:, 1:L], bprow[:, 1:L], 1.0)
                    nc.vector.memset(bprow[:, 0:1], 1.0)
                    msrow = cp.tile([1, L], F32, tag="msrow")
                    nc.vector.tensor_scalar(
                        msrow[:], bprow[:], 0.5, None, mybir.AluOpType.is_ge
                    )
                    urow = cp.tile([1, L], F32, tag="urow")
                    nc.vector.tensor_mul(urow[:], msrow[:], bprow[:])
                    arow = cp.tile([1, L], F32, tag="arow")
                    nc.vector.tensor_scalar(arow[:], urow[:], -1.0, 1.0, MULT, ADD)
                    crow = cp.tile([1, L], F32, tag="crow")
                    nc.vector.tensor_scalar_mul(
                        crow[:], urow[:], OUT_SCALE * CPT_SCALE
                    )

                    ones_col = cp.tile([1, P], F32, tag="ones")
                    nc.vector.memset(ones_col[:], 1.0)
                    for src, dst in [(arow, a_bc), (crow, coef_bc)]:
                        for j in range(L // 512):
                            ps = cps_.tile([P, 512], F32, tag="bcps")
                            nc.tensor.matmul(
                                ps[:], ones_col[:], src[:, j * 512 : (j + 1) * 512],
                                start=True, stop=True,
                            )
                            nc.vector.tensor_copy(
                                dst[:, j * 512 : (j + 1) * 512], ps[:]
                            )

                    # aux loss (R=2): aux = 2 - 2(F+G) + 4 F G
                    gs = cp.tile([1, 1], F32, tag="gs")
                    fs = cp.tile([1, 1], F32, tag="fs")
                    nc.vector.tensor_reduce(
                        gs[:], bprow[:], mybir.AxisListType.X, ADD
                    )
                    nc.vector.tensor_reduce(
                        fs[:], msrow[:], mybir.AxisListType.X, ADD
                    )
                    nc.vector.tensor_scalar_mul(gs[:], gs[:], 1.0 / L)
                    nc.vector.tensor_scalar_mul(fs[:], fs[:], 1.0 / L)
                    fg2 = cp.tile([1, 1], F32, tag="fg2")
                    nc.vector.tensor_mul(fg2[:], gs[:], fs[:])
                    sm = cp.tile([1, 1], F32, tag="sm")
                    nc.vector.tensor_add(sm[:], gs[:], fs[:])
                    nc.vector.tensor_scalar(sm[:], sm[:], -2.0, 2.0, MULT, ADD)
                    nc.vector.tensor_scalar(fg2[:], fg2[:], 4.0, None, MULT)
                    nc.vector.tensor_add(fg2[:], fg2[:], sm[:])
                    nc.sync.dma_start(out=aux_out[:], in_=fg2[:])

                # ---- stage D: dechunk scan, hidden-major ----
                with tc.tile_pool(name="dscan", bufs=2) as dp:
                    for k in range(NK):
                        ht = dp.tile([P, L], F32, tag="ht")
                        nc.sync.dma_start(out=ht[:], in_=hT[k * P : (k + 1) * P, :])
                        bt = dp.tile([P, L], F32, tag="bt")
                        nc.gpsimd.tensor_mul(bt[:], ht[:], coef_bc[:])
                        yt = dp.tile([P, L], F32, tag="yt")
                        nc.vector.tensor_tensor_scan(
                            out=yt[:], data0=a_bc[:], data1=bt[:], initial=0.0,
                            op0=MULT, op1=ADD,
                        )
                        ot = dp.tile([P, L], F32R, tag="ot")
                        nc.vector.scalar_tensor_tensor(
                            out=ot[:], in0=ht[:], scalar=OUT_SCALE, in1=yt[:],
                            op0=MULT, op1=ADD,
                        )
                        nc.sync.dma_start(
                            out=ohT[k * P : (k + 1) * P, :], in_=ot[:]
                        )

            # ---- stage E: lm_head in f32r, vocab-sharded ----
            NHALF = 2
            VH = VS // NHALF  # 2000
            NCH = 4
            VC = VH // NCH  # 500
            with (
                tc.tile_pool(name="lm_w", bufs=1) as lwp,
                tc.tile_pool(name="lm", bufs=1) as lp,
                tc.tile_pool(name="lm_ps", bufs=4, space="PSUM") as lps,
            ):
                for half in range(NHALF):
                    wfr = lwp.tile([P, NK * VH], F32R, tag="wfr")
                    for k in range(NK):
                        wst = lp.tile([P, VH], F32, tag="wst", bufs=2)
                        nc.sync.dma_start(
                            out=wst[:],
                            in_=lm_wT[
                                k * P : (k + 1) * P, half * VH : (half + 1) * VH
                            ],
                        )
                        nc.vector.tensor_copy(wfr[:, k * VH : (k + 1) * VH], wst[:])
                    for m in range(NSEQ):
                        oh = lp.tile([P, NK * P], F32R, tag="oh", bufs=2)
                        nc.sync.dma_start(
                            out=oh[:].rearrange("p (a b) -> p a b", a=NK),
                            in_=ohT[:].rearrange("(a p) t -> p a t", p=P)[
                                :, :, m * P : (m + 1) * P
                            ],
                        )
                        lg = lp.tile([P, VH], F32, tag="lg", bufs=2)
                        for n in range(NCH):
                            ps = lps.tile([P, VC], F32, tag="lps")
                            for k in range(NK):
                                nc.tensor.matmul(
                                    ps[:],
                                    oh[:, k * P : (k + 1) * P],
                                    wfr[
                                        :,
                                        k * VH + n * VC : k * VH + (n + 1) * VC,
                                    ],
                                    start=(k == 0),
                                    stop=(k == NK - 1),
                                )
                            nc.vector.tensor_copy(lg[:, n * VC : (n + 1) * VC], ps[:])
                        nc.sync.dma_start(
                            out=logits[
                                m * P : (m + 1) * P, half * VH : (half + 1) * VH
                            ],
                            in_=lg[:],
                        )

    _legalize_multi_waits(nc)
    return nc


_PROGRAM = None


def _get_program():
    global _PROGRAM
    if _PROGRAM is None:
        _PROGRAM = build_program()
    return _PROGRAM


def kernel(input_ids, emb, q_w, k_w, lm_w):
    input_ids = np.asarray(input_ids)
    emb = np.ascontiguousarray(np.asarray(emb, dtype=np.float32))
    q_w = np.asarray(q_w, dtype=np.float32)
    k_w = np.asarray(k_w, dtype=np.float32)
    lm_w = np.asarray(lm_w, dtype=np.float32)

    ids = input_ids.reshape(-1).astype(np.int64)
    assert ids.shape[0] == L

    uniq, inverse = np.unique(ids, return_inverse=True)
    tablearr = np.zeros((L, HD), np.float32)
    tablearr[: uniq.shape[0]] = emb[uniq]
    remaparr = inverse.astype(np.int32)

    q_wT = np.ascontiguousarray(q_w.T)
    k_wT = np.ascontiguousarray(k_w.T)
    lm_wT_full = np.ascontiguousarray(lm_w.T)  # [HD, V]

    in_maps = []
    for c in range(NCORES):
        span = remaparr[c * SEQ : min(c * SEQ + QKL, L)]
        qk = np.zeros(QKL, np.int32)
        qk[: span.shape[0]] = span
        in_maps.append(
            {
                "table": tablearr,
                "remap": remaparr,
                "qk_ids": qk,
                "q_wT": q_wT,
                "k_wT": k_wT,
                "lm_wT": np.ascontiguousarray(lm_wT_full[:, c * VS : (c + 1) * VS]),
            }
        )

    nc = _get_program()
    res = run_bass_kernel_spmd(nc, in_maps, list(range(NCORES)))
    logits = np.concatenate(
        [res.results[c]["logits"] for c in range(NCORES)], axis=1
    )[None, :, :]
    aux = np.float32(res.results[0]["aux"][0, 0])
    return logits, aux
